# revision 24
# baseline (speedup 1.0000x reference)
"""Fused sparse-attention kernel for Trainium2 (8 NeuronCores).

Computation (per batch element b):
    X[s,k]  = enc[b] @ W_enc + dec_proj[b,k] + cov[b,s]*Wcovsum[k] + bias[k]
    T       = tanh(X)
    att[s]  = T @ v_w                      (+ v_b, which cancels in softmax)
    w       = softmax(att masked to s < len[b])
    new_cov = cov + w

Key insight vs the batch-parallel baseline: positions s >= text_lengths[b]
are masked to -inf, so their softmax weight is exactly 0 and new_cov equals
cov there.  Only ceil(len_b/128) of the 16 s-tiles per batch need computing
(~55% on average for uniform lengths).  The device work unit is therefore a
flat list of (b, j) 128-position s-tiles packed by the host; tiles are dealt
round-robin to the 8 cores (a batch may straddle cores -- the softmax is a
host epilogue, so tiles are fully independent).  Every engine's load scales
with the masked tile count.

Per-tile pipeline (same numerics as the proven baseline):
  PE:  X psum[128s, 512k] = fp8 DoubleRow GEMM (enc fp8 *0.25, W_enc fp8 *16,
       net *4 undone by tanh's scale) + bf16 K=2 rank-1 (ones,cov) x (u_b, w)
  ACT: tanh over a whole 4-tile psum slot -> bf16 (one instr amortizes the
       ~185ns PSUM/SBUF access init)
  DVE: slot-wide tensor_tensor T*v (2x bf16), then per-tile tensor_scalar
       with accum_out for the free-dim reduce (4x mode)
PSUM rotates 2 slots x 4 banks.  Raw logits ship to the host, which does the
masked softmax (fp32, max-subtracted) + cov add on 65K values.

The Bass program depends only on NT (padded tiles per core), compiled on
first call per NT and cached; all (b,j) specifics live in host-packed input
blobs, so any text_lengths works.
"""

import numpy as np
import ml_dtypes

B, S, H, E = 32, 2048, 512, 512
NCORES = 8
HC = H // 128               # h chunks
BF16 = ml_dtypes.bfloat16
FP8 = ml_dtypes.float8_e4m3fn
ENC_SCALE = 0.25            # enc pre-scale (host)
W_SCALE = 16.0              # W_enc pre-scale (host)
PSUM_SCALE = ENC_SCALE * W_SCALE  # net scale on psum; undone in tanh

_CACHE = {}


def _build_nc(NT):
    """NT = padded tile count per core (multiple of 4)."""
    import concourse.mybir as mybir
    import concourse.tile as tile
    from concourse import bacc
    from contextlib import ExitStack

    dt = mybir.dt
    F32, BF = dt.float32, dt.bfloat16
    ENC_DT = dt.float8e4

    nc = bacc.Bacc("TRN2", target_bir_lowering=False, debug=False,
                   enable_asserts=False, num_devices=NCORES)

    # ---- DRAM I/O (per-core shapes) ----
    # encT[p, t*512 + c*128 + si] = enc[b_t, 128*j_t + si, 128*c + p] * 0.25
    # (fp8): per-partition 512B-contiguous runs per tile => full DMA rate
    encT = nc.dram_tensor("encT", [128, NT * HC * 128], ENC_DT,
                          kind="ExternalInput").ap()
    # wblob: wenc chunk c at cols [c*H,(c+1)*H): wenc[c][p,k] = W[128c+p, k]
    wblob = nc.dram_tensor("wblob", [128, HC * H], ENC_DT,
                           kind="ExternalInput").ap()
    # rank-1 blobs, fp8 DoubleRow layout (K=4 = 2 partitions x 2 DR rows) so a
    # tile can mix positions from two batches (the 32 partial batch tails are
    # bin-packed pairwise into shared tiles).  Per tile:
    #   lhs [2p, 2x, 128s] = ((m1, cov'*m1), (m2, cov'*m2)),  cov' = cov-0.5
    #   rhs [2p, 2x, 512k] = ((u_b1', w), (u_b2', w)) * PSUM_SCALE,
    #   u' = dec_proj + bias + 0.5*w  (the 0.5 undoes the cov centering)
    r1f = nc.dram_tensor("r1f", [2, NT * 2 * (128 + H)], ENC_DT,
                         kind="ExternalInput").ap()
    vbc = nc.dram_tensor("vbc", [128, HC], BF, kind="ExternalInput").ap()
    # raw attention logits, column t = tile t's 128 s-positions
    att_out = nc.dram_tensor("att_out", [128, NT], F32,
                             kind="ExternalOutput").ap()

    AF = mybir.ActivationFunctionType
    OP = mybir.AluOpType
    DR = mybir.MatmulPerfMode.DoubleRow

    # slot sizes: small slots at both ends shorten pipeline fill (first tanh
    # waits on a whole psum-tile slot: sync is tile-granular) and drain
    if NT >= 16:
        mid = NT - 8
        SLOTS = [1, 1, 2] + [4] * (mid // 4) + ([2] if mid % 4 else []) \
            + [2, 1, 1]
    else:
        SLOTS = [2] * (NT // 2)
    assert sum(SLOTS) == NT
    NSLOT = len(SLOTS)
    S_OFF = [sum(SLOTS[:i]) for i in range(NSLOT)]

    with tile.TileContext(nc) as tc, ExitStack() as ctx:
        consts = ctx.enter_context(tc.tile_pool(name="consts", bufs=1))
        encp = ctx.enter_context(tc.tile_pool(name="encp", bufs=6))
        tpool = ctx.enter_context(tc.tile_pool(name="tpool", bufs=3))
        spool = ctx.enter_context(tc.tile_pool(name="spool", bufs=3))
        s2pool = ctx.enter_context(tc.tile_pool(name="s2pool", bufs=3))
        ppm = ctx.enter_context(tc.tile_pool(name="ppm", bufs=2, space="PSUM"))

        # first-needed consts ride the SP HWDGE queue, smallest first; slot 0's
        # enc takes the Pool SWDGE path whose desc-gen overlaps the serialized
        # HWDGE issues.
        r1_sb = consts.tile([2, NT * 2 * (128 + H)], ENC_DT, tag="r1f")
        nc.sync.dma_start(r1_sb[:], r1f[:])
        wb_a = consts.tile([128, 2 * H], ENC_DT, tag="wb_a")
        nc.sync.dma_start(wb_a[:], wblob[:, 0:2 * H])
        wb_b = consts.tile([128, 2 * H], ENC_DT, tag="wb_b")
        nc.sync.dma_start(wb_b[:], wblob[:, 2 * H:])

        # PE p-state warm-up: junk matmuls on memset scratch keep the tensor
        # engine continuously busy from ~1us so the 3us ramp to full clock is
        # mostly done before the first real matmul (otherwise the whole
        # pipeline fill runs at the 2-4x slower mid/low p-state)
        warm = consts.tile([128, 256], BF, tag="warm")
        nc.gpsimd.memset(warm[:], 0.0)
        wps = ppm.tile([128, H], F32, tag="x")
        for _ in range(10):
            nc.tensor.matmul(wps[:, 0:256], warm[:, 0:128], warm[:],
                             start=True, stop=True)

        src_t = encT.rearrange("p (t x) -> p t x", t=NT)

        def enc_tile(n):
            return encp.tile([128, n, 128, HC], ENC_DT, tag="enc",
                             name="enc_t")

        def enc_load(g):
            e_t = enc_tile(SLOTS[g])
            lo = S_OFF[g]
            nc.sync.dma_start(
                e_t[:].rearrange("p q s x -> p q (s x)"),
                src_t[:, lo:lo + SLOTS[g], :])
            return e_t

        # slot 0's enc via Pool SWDGE
        e0 = enc_tile(SLOTS[0])
        nc.gpsimd.dma_start(e0[:].rearrange("p q s x -> p q (s x)"),
                            src_t[:, 0:SLOTS[0], :])
        PREFETCH = 5
        pre = {0: e0}
        for g in range(1, min(PREFETCH, NSLOT)):
            pre[g] = enc_load(g)

        vT_sb = consts.tile([128, HC], BF, tag="vbc")
        nc.gpsimd.dma_start(vT_sb[:], vbc[:])

        att_t = consts.tile([128, NT], F32, tag="att")

        r1l4 = r1_sb[:, 0:NT * 2 * 128] \
            .rearrange("p (t x c) -> p t x c", t=NT, x=2)
        r1r4 = r1_sb[:, NT * 2 * 128:] \
            .rearrange("p (t x k) -> p t x k", t=NT, x=2)
        wb3a = wb_a[:].rearrange("p (c k) -> p c k", c=2)
        wb3b = wb_b[:].rearrange("p (c k) -> p c k", c=2)

        # transposed-layout slot: psum bank kc holds X[k in chunk kc, s of the
        # slot's tiles].  After tanh consumes a slot's psum, the per-tile
        # v-dot matmuls (lhsT = tanh output T', rhs = v chunk, N=1: nearly
        # free on PE) accumulate the logit columns into the slot's own bank 0,
        # and one small DVE copy moves them to SBUF.  DVE is otherwise idle.
        prev = None  # (ps, t_t, T0, NQ) of the previous slot, for vdots

        def emit_vdots(ps, t_t, T0, NQ):
            t4 = t_t[:].rearrange("p (c y) -> p c y", c=HC)
            for q in range(NQ):
                for kc in range(HC):
                    nc.tensor.matmul(
                        ps[:, q:q + 1],
                        t4[:, kc, q * 128:(q + 1) * 128],
                        vT_sb[:, kc:kc + 1],
                        start=(kc == 0), stop=(kc == HC - 1),
                    )
            nc.vector.tensor_copy(att_t[:, T0:T0 + NQ], ps[:, 0:NQ])

        for g in range(NSLOT):
            NQ = SLOTS[g]
            T0 = S_OFF[g]
            enc_t = pre.pop(g)
            if g + PREFETCH < NSLOT:
                pre[g + PREFETCH] = enc_load(g + PREFETCH)

            ps = ppm.tile([128, 4 * H], F32, tag="x")
            # rank-1s first: they only need the small r1 blob. Only the first
            # matmul into each bank uses start=True (start clears the whole
            # bank's has_written bits); later rank-1s overwrite their own
            # fresh columns, mains accumulate.
            for kc in range(HC):
                for q in range(NQ):
                    t = T0 + q
                    nc.tensor.matmul(
                        ps[:, kc * H + q * 128:kc * H + (q + 1) * 128],
                        r1r4[:, t, :, kc * 128:(kc + 1) * 128],
                        r1l4[:, t],
                        start=(q == 0), stop=False, perf_mode=DR,
                    )
            for kc in range(HC):
                psl = ps[:, kc * H:kc * H + NQ * 128]
                for pp, wb in ((0, wb3a), (1, wb3b)):
                    nc.tensor.matmul(
                        psl,
                        wb[:, 0:2, kc * 128:(kc + 1) * 128],
                        enc_t[:, 0:NQ, :, 2 * pp:2 * pp + 2]
                            .rearrange("p q s x -> p x (q s)"),
                        start=False, stop=(pp == 1),
                        perf_mode=DR,
                    )
            if prev is not None:
                emit_vdots(*prev)

            if g == NSLOT - 1:
                pieces = [(0, NQ - 1), (NQ - 1, 1)] if NQ > 1 else [(0, 1)]
            else:
                pieces = [(0, NQ)]
            t_t = tpool.tile([128, NQ * 4 * 128], BF, tag="t")
            ps4 = ps[:].rearrange("p (c y) -> p c y", c=HC)
            t4 = t_t[:].rearrange("p (c y) -> p c y", c=HC)
            for g0, glen in pieces:
                nc.scalar.activation(
                    t4[:, :, g0 * 128:(g0 + glen) * 128],
                    ps4[:, :, g0 * 128:(g0 + glen) * 128],
                    AF.Tanh, scale=1.0 / PSUM_SCALE)
            prev = (ps, t_t, T0, NQ)

            # ship logits in pieces so only the last columns' DMA trails the
            # final compute (vdots/copy for slot g are emitted at g+1, so the
            # covered columns end at the previous slot)
            if g == NSLOT - 3:
                nc.sync.dma_start(att_out[:, 0:T0], att_t[:, 0:T0])
            elif g == NSLOT - 1:
                lo = S_OFF[NSLOT - 3]
                nc.sync.dma_start(att_out[:, lo:T0], att_t[:, lo:T0])
        emit_vdots(*prev)
        lastT = S_OFF[NSLOT - 1]
        nc.sync.dma_start(att_out[:, lastT:], att_t[:, lastT:])

    nc.compile()
    return nc


def _get_nc(NT=None):
    if NT is None:
        NT = _CACHE.get("last_nt")
        assert NT is not None, "call kernel() first"
    if ("nc", NT) not in _CACHE:
        _CACHE[("nc", NT)] = _build_nc(NT)
    _CACHE["last_nt"] = NT
    return _CACHE[("nc", NT)]


def _prep(dec_input, enc_output, text_lengths, coverage_vector, W, b, v_w):
    enc = np.asarray(enc_output, dtype=np.float32)
    dec = np.asarray(dec_input, dtype=np.float32).reshape(B, E)
    cov = np.asarray(coverage_vector, dtype=np.float32)
    W = np.asarray(W, dtype=np.float32)
    b = np.asarray(b, dtype=np.float32)
    v_w = np.asarray(v_w, dtype=np.float32)
    lens = np.asarray(text_lengths).astype(np.int64)

    # ---- position-level tile packing ----
    # Full 128-position tiles per batch; the 32 partial tails are bin-packed
    # pairwise (<=2 batch segments per tile, handled by the K=4 rank-1).
    nfull = (lens // 128).astype(int)
    tail = (lens - nfull * 128).astype(int)

    tiles = []  # each: list of (b, s_start, n)
    for bi in range(B):
        for j in range(nfull[bi]):
            tiles.append([(bi, j * 128, 128)])
    tails = sorted([(int(tail[bi]), bi) for bi in range(B) if tail[bi] > 0],
                   reverse=True)
    open_tiles = []  # (free, idx)
    for n, bi in tails:
        placed = False
        for k, (free, idx) in enumerate(open_tiles):
            if free >= n and len(tiles[idx]) < 2:
                tiles[idx].append((bi, nfull[bi] * 128, n))
                open_tiles[k] = (free - n, idx)
                placed = True
                break
        if not placed:
            tiles.append([(bi, nfull[bi] * 128, n)])
            open_tiles.append((128 - n, len(tiles) - 1))

    T_real = len(tiles)
    NT = max(16, -(-T_real // NCORES))
    NT += NT % 2                          # slot pattern needs an even count
    T_pad = NT * NCORES
    tiles += [[(0, 0, 128)]] * (T_pad - T_real)

    # position-level index arrays
    bb = np.zeros((T_pad, 128), np.int64)     # batch of each position
    ss = np.zeros((T_pad, 128), np.int64)     # s of each position
    valid = np.zeros((T_pad, 128), bool)
    seg2 = np.zeros((T_pad, 128), bool)       # position belongs to segment 2
    b1 = np.zeros(T_pad, np.int64)
    b2 = np.zeros(T_pad, np.int64)
    for t, segs in enumerate(tiles):
        off = 0
        b1[t] = segs[0][0]
        b2[t] = segs[-1][0]
        for si, (bi, s0, n) in enumerate(segs):
            bb[t, off:off + n] = bi
            ss[t, off:off + n] = s0 + np.arange(n)
            valid[t, off:off + n] = True
            if si == 1:
                seg2[t, off:off + n] = True
            off += n

    # enc gather+transpose for selected positions only:
    # block[p, t, c*128+si] = enc[bb[t,si], ss[t,si], 128c+p] * ENC_SCALE
    blk = enc[bb, ss]                           # [T, si, h]
    blk = (blk * ENC_SCALE).astype(FP8).reshape(T_pad, 128, HC, 128)
    # encT[p, t, s, c] = enc[bb, ss, 128c+p]: transposed-GEMM moving operand
    encT = np.ascontiguousarray(blk.transpose(3, 0, 1, 2)) \
        .reshape(128, T_pad * HC * 128)

    wenc = W[:H] * W_SCALE
    wblob = np.ascontiguousarray(
        wenc.reshape(HC, 128, H).transpose(1, 0, 2).reshape(128, HC * H)
    ).astype(FP8)

    dec_proj = dec @ W[H:H + E] + b                    # (B, H)
    wcovsum = W[H + E:].sum(axis=0, dtype=np.float32)  # (H,)

    # rank-1 blobs, fp8 DR: cov centered at 0.5 to halve fp8 quantization
    # error; the 0.5*w shift folds into u (kept exact in fp32 before cast)
    covp = cov[bb, ss] - 0.5
    m1 = (valid & ~seg2).astype(np.float32)
    m2 = (valid & seg2).astype(np.float32)
    r1l = np.empty((2, T_pad, 2, 128), np.float32)
    r1l[0, :, 0] = m1
    r1l[0, :, 1] = covp * m1
    r1l[1, :, 0] = m2
    r1l[1, :, 1] = covp * m2
    uu = (dec_proj + 0.5 * wcovsum) * PSUM_SCALE       # (B, H)
    r1r = np.empty((2, T_pad, 2, H), np.float32)
    r1r[0, :, 0] = uu[b1]
    r1r[0, :, 1] = wcovsum * PSUM_SCALE
    r1r[1, :, 0] = uu[b2]
    r1r[1, :, 1] = wcovsum * PSUM_SCALE
    r1l = r1l.astype(FP8)
    r1r = r1r.astype(FP8)

    vbc = np.ascontiguousarray(v_w.reshape(HC, 128).T.astype(BF16))

    in_maps = []
    for core in range(NCORES):
        sl = slice(core * NT, (core + 1) * NT)
        in_maps.append({
            "encT": np.ascontiguousarray(
                encT.reshape(128, T_pad, HC * 128)[:, sl]
                .reshape(128, NT * HC * 128)),
            "wblob": wblob,
            "r1f": np.ascontiguousarray(np.concatenate(
                [r1l[:, sl].reshape(2, NT * 2 * 128),
                 r1r[:, sl].reshape(2, NT * 2 * H)], axis=1)),
            "vbc": vbc,
        })
    return in_maps, bb, ss, valid, NT


def kernel(dec_input, enc_output, text_lengths, coverage_vector, W, b, v_w, v_b):
    from concourse.bass_utils import run_bass_kernel_spmd

    in_maps, bb, ss, valid, NT = _prep(
        dec_input, enc_output, text_lengths, coverage_vector, W, b, v_w)
    nc = _get_nc(NT)
    res = run_bass_kernel_spmd(nc, in_maps, core_ids=list(range(NCORES)))

    # scatter raw logits back to (b, s); untouched positions stay -inf
    logits = np.full((B, S), -np.inf, np.float32)
    cols = np.concatenate([res.results[c]["att_out"].T for c in range(NCORES)],
                          axis=0)                     # [T_pad, 128]
    logits[bb[valid], ss[valid]] = cols[valid]
    # masked softmax epilogue (full fp32, max-subtracted)
    lens = np.asarray(text_lengths).reshape(B, 1)
    masked = np.where(np.arange(S)[None, :] < lens, logits, -np.inf)
    masked -= masked.max(axis=1, keepdims=True)
    att = np.exp(masked)
    att /= att.sum(axis=1, keepdims=True, dtype=np.float32)
    ncov = np.asarray(coverage_vector, dtype=np.float32) + att
    return att, ncov


# revision 25
# speedup vs baseline: 1.0460x; 1.0460x over previous
"""Fused sparse-attention kernel for Trainium2 (8 NeuronCores).

Computation (per batch element b):
    X[s,k]  = enc[b] @ W_enc + dec_proj[b,k] + cov[b,s]*Wcovsum[k] + bias[k]
    T       = tanh(X)
    att[s]  = T @ v_w                      (+ v_b, which cancels in softmax)
    w       = softmax(att masked to s < len[b])
    new_cov = cov + w

Key insight vs the batch-parallel baseline: positions s >= text_lengths[b]
are masked to -inf, so their softmax weight is exactly 0 and new_cov equals
cov there.  Only ceil(len_b/128) of the 16 s-tiles per batch need computing
(~55% on average for uniform lengths).  The device work unit is therefore a
flat list of (b, j) 128-position s-tiles packed by the host; tiles are dealt
round-robin to the 8 cores (a batch may straddle cores -- the softmax is a
host epilogue, so tiles are fully independent).  Every engine's load scales
with the masked tile count.

Per-tile pipeline (same numerics as the proven baseline):
  PE:  X psum[128s, 512k] = fp8 DoubleRow GEMM (enc fp8 *0.25, W_enc fp8 *16,
       net *4 undone by tanh's scale) + bf16 K=2 rank-1 (ones,cov) x (u_b, w)
  ACT: tanh over a whole 4-tile psum slot -> bf16 (one instr amortizes the
       ~185ns PSUM/SBUF access init)
  DVE: slot-wide tensor_tensor T*v (2x bf16), then per-tile tensor_scalar
       with accum_out for the free-dim reduce (4x mode)
PSUM rotates 2 slots x 4 banks.  Raw logits ship to the host, which does the
masked softmax (fp32, max-subtracted) + cov add on 65K values.

The Bass program depends only on NT (padded tiles per core), compiled on
first call per NT and cached; all (b,j) specifics live in host-packed input
blobs, so any text_lengths works.
"""

import numpy as np
import ml_dtypes

B, S, H, E = 32, 2048, 512, 512
NCORES = 8
HC = H // 128               # h chunks
BF16 = ml_dtypes.bfloat16
FP8 = ml_dtypes.float8_e4m3fn
ENC_SCALE = 0.25            # enc pre-scale (host)
W_SCALE = 16.0              # W_enc pre-scale (host)
PSUM_SCALE = ENC_SCALE * W_SCALE  # net scale on psum; undone in tanh

_CACHE = {}


def _build_nc(NT):
    """NT = padded tile count per core (multiple of 4)."""
    import concourse.mybir as mybir
    import concourse.tile as tile
    from concourse import bacc
    from contextlib import ExitStack

    dt = mybir.dt
    F32, BF = dt.float32, dt.bfloat16
    ENC_DT = dt.float8e4

    nc = bacc.Bacc("TRN2", target_bir_lowering=False, debug=False,
                   enable_asserts=False, num_devices=NCORES)

    # ---- DRAM I/O (per-core shapes) ----
    # encT[p, t*512 + c*128 + si] = enc[b_t, 128*j_t + si, 128*c + p] * 0.25
    # (fp8): per-partition 512B-contiguous runs per tile => full DMA rate
    encT = nc.dram_tensor("encT", [128, NT * HC * 128], ENC_DT,
                          kind="ExternalInput").ap()
    # wblob: wenc chunk c at cols [c*H,(c+1)*H): wenc[c][p,k] = W[128c+p, k]
    wblob = nc.dram_tensor("wblob", [128, HC * H], ENC_DT,
                           kind="ExternalInput").ap()
    # rank-1 blobs, fp8 DoubleRow layout (K=4 = 2 partitions x 2 DR rows) so a
    # tile can mix positions from two batches (the 32 partial batch tails are
    # bin-packed pairwise into shared tiles).  Per tile:
    #   lhs [2p, 2x, 128s] = ((m1, cov'*m1), (m2, cov'*m2)),  cov' = cov-0.5
    #   rhs [2p, 2x, 512k] = ((u_b1', w), (u_b2', w)) * PSUM_SCALE,
    #   u' = dec_proj + bias + 0.5*w  (the 0.5 undoes the cov centering)
    r1f = nc.dram_tensor("r1f", [2, NT * 2 * (128 + H)], ENC_DT,
                         kind="ExternalInput").ap()
    vbc = nc.dram_tensor("vbc", [128, H], BF, kind="ExternalInput").ap()
    # raw attention logits, column t = tile t's 128 s-positions
    att_out = nc.dram_tensor("att_out", [128, NT], F32,
                             kind="ExternalOutput").ap()

    AF = mybir.ActivationFunctionType
    OP = mybir.AluOpType
    DR = mybir.MatmulPerfMode.DoubleRow

    # slot sizes: small slots at both ends shorten pipeline fill (first tanh
    # waits on a whole psum-tile slot: sync is tile-granular) and drain
    if NT >= 16:
        mid = NT - 8
        SLOTS = [1, 1, 2] + [4] * (mid // 4) + ([2] if mid % 4 else []) \
            + [2, 1, 1]
    else:
        SLOTS = [2] * (NT // 2)
    assert sum(SLOTS) == NT
    NSLOT = len(SLOTS)
    S_OFF = [sum(SLOTS[:i]) for i in range(NSLOT)]

    with tile.TileContext(nc) as tc, ExitStack() as ctx:
        consts = ctx.enter_context(tc.tile_pool(name="consts", bufs=1))
        encp = ctx.enter_context(tc.tile_pool(name="encp", bufs=6))
        tpool = ctx.enter_context(tc.tile_pool(name="tpool", bufs=3))
        spool = ctx.enter_context(tc.tile_pool(name="spool", bufs=3))
        s2pool = ctx.enter_context(tc.tile_pool(name="s2pool", bufs=3))
        ppm = ctx.enter_context(tc.tile_pool(name="ppm", bufs=2, space="PSUM"))

        # first-needed consts ride the SP HWDGE queue, smallest first; slot 0's
        # enc takes the Pool SWDGE path whose desc-gen overlaps the serialized
        # HWDGE issues.
        r1_sb = consts.tile([2, NT * 2 * (128 + H)], ENC_DT, tag="r1f")
        nc.sync.dma_start(r1_sb[:], r1f[:])
        wb_a = consts.tile([128, 2 * H], ENC_DT, tag="wb_a")
        nc.sync.dma_start(wb_a[:], wblob[:, 0:2 * H])
        wb_b = consts.tile([128, 2 * H], ENC_DT, tag="wb_b")
        nc.sync.dma_start(wb_b[:], wblob[:, 2 * H:])

        # PE p-state warm-up: junk matmuls on memset scratch keep the tensor
        # engine continuously busy from ~1us so the 3us ramp to full clock is
        # mostly done before the first real matmul (otherwise the whole
        # pipeline fill runs at the 2-4x slower mid/low p-state)
        warm = consts.tile([128, 256], BF, tag="warm")
        nc.gpsimd.memset(warm[:], 0.0)
        wps = ppm.tile([128, H], F32, tag="x")
        for _ in range(10):
            nc.tensor.matmul(wps[:, 0:256], warm[:, 0:128], warm[:],
                             start=True, stop=True)

        src_t = encT.rearrange("p (t x) -> p t x", t=NT)

        def enc_tile(n):
            return encp.tile([128, n, HC * 128], ENC_DT, tag="enc",
                             name="enc_t")

        def enc_load(g):
            e_t = enc_tile(SLOTS[g])
            lo = S_OFF[g]
            nc.sync.dma_start(e_t[:], src_t[:, lo:lo + SLOTS[g], :])
            return e_t

        # slot 0's enc via Pool SWDGE
        e0 = enc_tile(SLOTS[0])
        nc.gpsimd.dma_start(e0[:], src_t[:, 0:SLOTS[0], :])
        PREFETCH = 5
        pre = {0: e0}
        for g in range(1, min(PREFETCH, NSLOT)):
            pre[g] = enc_load(g)

        vbc_sb = consts.tile([128, H], BF, tag="vbc")
        nc.gpsimd.dma_start(vbc_sb[:], vbc[:])

        att_t = consts.tile([128, NT], F32, tag="att")

        r1l4 = r1_sb[:, 0:NT * 2 * 128] \
            .rearrange("p (t x c) -> p t x c", t=NT, x=2)
        r1r4 = r1_sb[:, NT * 2 * 128:] \
            .rearrange("p (t x k) -> p t x k", t=NT, x=2)
        wb3a = wb_a[:].rearrange("p (c k) -> p c k", c=2)
        wb3b = wb_b[:].rearrange("p (c k) -> p c k", c=2)

        for g in range(NSLOT):
            NQ = SLOTS[g]
            T0 = S_OFF[g]
            enc_t = pre.pop(g)
            if g + PREFETCH < NSLOT:
                pre[g + PREFETCH] = enc_load(g + PREFETCH)

            enc4 = enc_t[:].rearrange("p q (c y) -> p q c y", c=HC)
            ps = ppm.tile([128, NQ * H], F32, tag="x")
            # rank-1s of all tiles first: they depend only on the small r1
            # blob, so at the head PE starts (and ramps) before enc lands
            for q in range(NQ):
                t = T0 + q
                nc.tensor.matmul(
                    ps[:, q * H:(q + 1) * H],
                    r1l4[:, t],
                    r1r4[:, t],
                    start=True, stop=False, perf_mode=DR,
                )
            for q in range(NQ):
                psl = ps[:, q * H:(q + 1) * H]
                for c, wb in ((0, wb3a), (2, wb3b)):
                    nc.tensor.matmul(
                        psl,
                        enc4[:, q, c:c + 2, :],
                        wb[:, 0:2, :],
                        start=False, stop=(c + 2 == HC),
                        perf_mode=DR,
                    )

            # ACT/DVE granularity: whole slot (one tanh / one mult instr
            # amortizes the access-latency init over the slot)
            if g == NSLOT - 1:
                pieces = [(0, NQ - 1), (NQ - 1, 1)] if NQ > 1 else [(0, 1)]
            else:
                pieces = [(0, NQ)]
            t_t = tpool.tile([128, NQ * H], BF, tag="t")
            scr = spool.tile([128, NQ * H], BF, tag="scr")
            for g0, glen in pieces:
                sl = slice(g0 * H, (g0 + glen) * H)
                nc.scalar.activation(t_t[:, sl], ps[:, sl], AF.Tanh,
                                     scale=1.0 / PSUM_SCALE)
                vb = vbc_sb[:].unsqueeze(1).broadcast_to([128, glen, H])
                nc.vector.tensor_tensor(
                    scr[:, sl].rearrange("p (q k) -> p q k", q=glen),
                    t_t[:, sl].rearrange("p (q k) -> p q k", q=glen),
                    vb, OP.mult)
                for q in range(g0, g0 + glen):
                    t = T0 + q
                    scr2 = s2pool.tile([128, H], BF, tag="scr2")
                    nc.vector.tensor_scalar(
                        scr2[:], scr[:, q * H:(q + 1) * H], 1.0, None,
                        OP.mult, OP.add, accum_out=att_t[:, t:t + 1],
                    )
            # ship logits in pieces so only the last columns' DMA trails the
            # final compute
            if g == NSLOT - 4:
                nc.sync.dma_start(att_out[:, 0:T0 + NQ], att_t[:, 0:T0 + NQ])
            elif g == NSLOT - 2:
                nc.sync.dma_start(att_out[:, S_OFF[NSLOT - 4] + SLOTS[NSLOT - 4]:T0 + NQ],
                                  att_t[:, S_OFF[NSLOT - 4] + SLOTS[NSLOT - 4]:T0 + NQ])
        lastT = S_OFF[NSLOT - 1]
        nc.sync.dma_start(att_out[:, lastT:], att_t[:, lastT:])

    nc.compile()
    return nc


def _get_nc(NT=None):
    if NT is None:
        NT = _CACHE.get("last_nt")
        assert NT is not None, "call kernel() first"
    if ("nc", NT) not in _CACHE:
        _CACHE[("nc", NT)] = _build_nc(NT)
    _CACHE["last_nt"] = NT
    return _CACHE[("nc", NT)]


def _prep(dec_input, enc_output, text_lengths, coverage_vector, W, b, v_w):
    enc = np.asarray(enc_output, dtype=np.float32)
    dec = np.asarray(dec_input, dtype=np.float32).reshape(B, E)
    cov = np.asarray(coverage_vector, dtype=np.float32)
    W = np.asarray(W, dtype=np.float32)
    b = np.asarray(b, dtype=np.float32)
    v_w = np.asarray(v_w, dtype=np.float32)
    lens = np.asarray(text_lengths).astype(np.int64)

    # ---- position-level tile packing ----
    # Full 128-position tiles per batch; the 32 partial tails are bin-packed
    # pairwise (<=2 batch segments per tile, handled by the K=4 rank-1).
    nfull = (lens // 128).astype(int)
    tail = (lens - nfull * 128).astype(int)

    tiles = []  # each: list of (b, s_start, n)
    for bi in range(B):
        for j in range(nfull[bi]):
            tiles.append([(bi, j * 128, 128)])
    tails = sorted([(int(tail[bi]), bi) for bi in range(B) if tail[bi] > 0],
                   reverse=True)
    open_tiles = []  # (free, idx)
    for n, bi in tails:
        placed = False
        for k, (free, idx) in enumerate(open_tiles):
            if free >= n and len(tiles[idx]) < 2:
                tiles[idx].append((bi, nfull[bi] * 128, n))
                open_tiles[k] = (free - n, idx)
                placed = True
                break
        if not placed:
            tiles.append([(bi, nfull[bi] * 128, n)])
            open_tiles.append((128 - n, len(tiles) - 1))

    T_real = len(tiles)
    NT = max(16, -(-T_real // NCORES))
    NT += NT % 2                          # slot pattern needs an even count
    T_pad = NT * NCORES
    tiles += [[(0, 0, 128)]] * (T_pad - T_real)

    # position-level index arrays
    bb = np.zeros((T_pad, 128), np.int64)     # batch of each position
    ss = np.zeros((T_pad, 128), np.int64)     # s of each position
    valid = np.zeros((T_pad, 128), bool)
    seg2 = np.zeros((T_pad, 128), bool)       # position belongs to segment 2
    b1 = np.zeros(T_pad, np.int64)
    b2 = np.zeros(T_pad, np.int64)
    for t, segs in enumerate(tiles):
        off = 0
        b1[t] = segs[0][0]
        b2[t] = segs[-1][0]
        for si, (bi, s0, n) in enumerate(segs):
            bb[t, off:off + n] = bi
            ss[t, off:off + n] = s0 + np.arange(n)
            valid[t, off:off + n] = True
            if si == 1:
                seg2[t, off:off + n] = True
            off += n

    # enc gather+transpose for selected positions only:
    # block[p, t, c*128+si] = enc[bb[t,si], ss[t,si], 128c+p] * ENC_SCALE
    blk = enc[bb, ss]                           # [T, si, h]
    blk = (blk * ENC_SCALE).astype(FP8).reshape(T_pad, 128, HC, 128)
    encT = np.ascontiguousarray(blk.transpose(3, 0, 2, 1)) \
        .reshape(128, T_pad * HC * 128)

    wenc = W[:H] * W_SCALE
    wblob = np.ascontiguousarray(
        wenc.reshape(HC, 128, H).transpose(1, 0, 2).reshape(128, HC * H)
    ).astype(FP8)

    dec_proj = dec @ W[H:H + E] + b                    # (B, H)
    wcovsum = W[H + E:].sum(axis=0, dtype=np.float32)  # (H,)

    # rank-1 blobs, fp8 DR: cov centered at 0.5 to halve fp8 quantization
    # error; the 0.5*w shift folds into u (kept exact in fp32 before cast)
    covp = cov[bb, ss] - 0.5
    m1 = (valid & ~seg2).astype(np.float32)
    m2 = (valid & seg2).astype(np.float32)
    r1l = np.empty((2, T_pad, 2, 128), np.float32)
    r1l[0, :, 0] = m1
    r1l[0, :, 1] = covp * m1
    r1l[1, :, 0] = m2
    r1l[1, :, 1] = covp * m2
    uu = (dec_proj + 0.5 * wcovsum) * PSUM_SCALE       # (B, H)
    r1r = np.empty((2, T_pad, 2, H), np.float32)
    r1r[0, :, 0] = uu[b1]
    r1r[0, :, 1] = wcovsum * PSUM_SCALE
    r1r[1, :, 0] = uu[b2]
    r1r[1, :, 1] = wcovsum * PSUM_SCALE
    r1l = r1l.astype(FP8)
    r1r = r1r.astype(FP8)

    vbc = np.ascontiguousarray(np.broadcast_to(v_w.astype(BF16), (128, H)))

    in_maps = []
    for core in range(NCORES):
        sl = slice(core * NT, (core + 1) * NT)
        in_maps.append({
            "encT": np.ascontiguousarray(
                encT.reshape(128, T_pad, HC * 128)[:, sl]
                .reshape(128, NT * HC * 128)),
            "wblob": wblob,
            "r1f": np.ascontiguousarray(np.concatenate(
                [r1l[:, sl].reshape(2, NT * 2 * 128),
                 r1r[:, sl].reshape(2, NT * 2 * H)], axis=1)),
            "vbc": vbc,
        })
    return in_maps, bb, ss, valid, NT


def kernel(dec_input, enc_output, text_lengths, coverage_vector, W, b, v_w, v_b):
    from concourse.bass_utils import run_bass_kernel_spmd

    in_maps, bb, ss, valid, NT = _prep(
        dec_input, enc_output, text_lengths, coverage_vector, W, b, v_w)
    nc = _get_nc(NT)
    res = run_bass_kernel_spmd(nc, in_maps, core_ids=list(range(NCORES)))

    # scatter raw logits back to (b, s); untouched positions stay -inf
    logits = np.full((B, S), -np.inf, np.float32)
    cols = np.concatenate([res.results[c]["att_out"].T for c in range(NCORES)],
                          axis=0)                     # [T_pad, 128]
    logits[bb[valid], ss[valid]] = cols[valid]
    # masked softmax epilogue (full fp32, max-subtracted)
    lens = np.asarray(text_lengths).reshape(B, 1)
    masked = np.where(np.arange(S)[None, :] < lens, logits, -np.inf)
    masked -= masked.max(axis=1, keepdims=True)
    att = np.exp(masked)
    att /= att.sum(axis=1, keepdims=True, dtype=np.float32)
    ncov = np.asarray(coverage_vector, dtype=np.float32) + att
    return att, ncov


# revision 27
# speedup vs baseline: 1.0566x; 1.0102x over previous
"""Fused sparse-attention kernel for Trainium2 (8 NeuronCores).

Computation (per batch element b):
    X[s,k]  = enc[b] @ W_enc + dec_proj[b,k] + cov[b,s]*Wcovsum[k] + bias[k]
    T       = tanh(X)
    att[s]  = T @ v_w                      (+ v_b, which cancels in softmax)
    w       = softmax(att masked to s < len[b])
    new_cov = cov + w

Key insight vs the batch-parallel baseline: positions s >= text_lengths[b]
are masked to -inf, so their softmax weight is exactly 0 and new_cov equals
cov there.  Only ceil(len_b/128) of the 16 s-tiles per batch need computing
(~55% on average for uniform lengths).  The device work unit is therefore a
flat list of (b, j) 128-position s-tiles packed by the host; tiles are dealt
round-robin to the 8 cores (a batch may straddle cores -- the softmax is a
host epilogue, so tiles are fully independent).  Every engine's load scales
with the masked tile count.

Per-tile pipeline (same numerics as the proven baseline):
  PE:  X psum[128s, 512k] = fp8 DoubleRow GEMM (enc fp8 *0.25, W_enc fp8 *16,
       net *4 undone by tanh's scale) + bf16 K=2 rank-1 (ones,cov) x (u_b, w)
  ACT: tanh over a whole 4-tile psum slot -> bf16 (one instr amortizes the
       ~185ns PSUM/SBUF access init)
  DVE: slot-wide tensor_tensor T*v (2x bf16), then per-tile tensor_scalar
       with accum_out for the free-dim reduce (4x mode)
PSUM rotates 2 slots x 4 banks.  Raw logits ship to the host, which does the
masked softmax (fp32, max-subtracted) + cov add on 65K values.

The Bass program depends only on NT (padded tiles per core), compiled on
first call per NT and cached; all (b,j) specifics live in host-packed input
blobs, so any text_lengths works.
"""

import numpy as np
import ml_dtypes

B, S, H, E = 32, 2048, 512, 512
NCORES = 8
HC = H // 128               # h chunks
BF16 = ml_dtypes.bfloat16
FP8 = ml_dtypes.float8_e4m3fn
ENC_SCALE = 0.25            # enc pre-scale (host)
W_SCALE = 16.0              # W_enc pre-scale (host)
PSUM_SCALE = ENC_SCALE * W_SCALE  # net scale on psum; undone in tanh

_CACHE = {}


def _build_nc(NT):
    """NT = padded tile count per core (multiple of 4)."""
    import concourse.mybir as mybir
    import concourse.tile as tile
    from concourse import bacc
    from contextlib import ExitStack

    dt = mybir.dt
    F32, BF = dt.float32, dt.bfloat16
    ENC_DT = dt.float8e4

    nc = bacc.Bacc("TRN2", target_bir_lowering=False, debug=False,
                   enable_asserts=False, num_devices=NCORES)

    # ---- DRAM I/O (per-core shapes) ----
    # encT[p, t*512 + c*128 + si] = enc[b_t, 128*j_t + si, 128*c + p] * 0.25
    # (fp8): per-partition 512B-contiguous runs per tile => full DMA rate
    encT = nc.dram_tensor("encT", [128, NT * HC * 128], ENC_DT,
                          kind="ExternalInput").ap()
    # wblob: wenc chunk c at cols [c*H,(c+1)*H): wenc[c][p,k] = W[128c+p, k]
    wblob = nc.dram_tensor("wblob", [128, HC * H], ENC_DT,
                           kind="ExternalInput").ap()
    # rank-1 blobs, fp8 DoubleRow layout (K=4 = 2 partitions x 2 DR rows) so a
    # tile can mix positions from two batches (the 32 partial batch tails are
    # bin-packed pairwise into shared tiles).  Per tile:
    #   lhs [2p, 2x, 128s] = ((m1, cov'*m1), (m2, cov'*m2)),  cov' = cov-0.5
    #   rhs [2p, 2x, 512k] = ((u_b1', w), (u_b2', w)) * PSUM_SCALE,
    #   u' = dec_proj + bias + 0.5*w  (the 0.5 undoes the cov centering)
    r1f = nc.dram_tensor("r1f", [2, NT * 2 * (128 + H)], ENC_DT,
                         kind="ExternalInput").ap()
    vbc = nc.dram_tensor("vbc", [128, H], BF, kind="ExternalInput").ap()
    # raw attention logits, column t = tile t's 128 s-positions
    att_out = nc.dram_tensor("att_out", [128, NT], F32,
                             kind="ExternalOutput").ap()

    AF = mybir.ActivationFunctionType
    OP = mybir.AluOpType
    DR = mybir.MatmulPerfMode.DoubleRow

    # slot sizes: small slots at both ends shorten pipeline fill (first tanh
    # waits on a whole psum-tile slot: sync is tile-granular) and drain
    if NT >= 16:
        mid = NT - 8
        SLOTS = [1, 1, 2] + [4] * (mid // 4) + ([2] if mid % 4 else []) \
            + [2, 1, 1]
    else:
        SLOTS = [2] * (NT // 2)
    assert sum(SLOTS) == NT
    NSLOT = len(SLOTS)
    S_OFF = [sum(SLOTS[:i]) for i in range(NSLOT)]

    with tile.TileContext(nc) as tc, ExitStack() as ctx:
        consts = ctx.enter_context(tc.tile_pool(name="consts", bufs=1))
        encp = ctx.enter_context(tc.tile_pool(name="encp", bufs=6))
        tpool = ctx.enter_context(tc.tile_pool(name="tpool", bufs=3))
        spool = ctx.enter_context(tc.tile_pool(name="spool", bufs=3))
        s2pool = ctx.enter_context(tc.tile_pool(name="s2pool", bufs=3))
        ppm = ctx.enter_context(tc.tile_pool(name="ppm", bufs=2, space="PSUM"))

        # first-needed consts ride the SP HWDGE queue, smallest first; slot 0's
        # enc takes the Pool SWDGE path whose desc-gen overlaps the serialized
        # HWDGE issues.
        r1_sb = consts.tile([2, NT * 2 * (128 + H)], ENC_DT, tag="r1f")
        nc.sync.dma_start(r1_sb[:], r1f[:])
        wb_a = consts.tile([128, 2 * H], ENC_DT, tag="wb_a")
        nc.sync.dma_start(wb_a[:], wblob[:, 0:2 * H])
        wb_b = consts.tile([128, 2 * H], ENC_DT, tag="wb_b")
        nc.sync.dma_start(wb_b[:], wblob[:, 2 * H:])

        # PE p-state warm-up: junk matmuls on memset scratch keep the tensor
        # engine continuously busy from ~1us so the 3us ramp to full clock is
        # mostly done before the first real matmul (otherwise the whole
        # pipeline fill runs at the 2-4x slower mid/low p-state)
        warm = consts.tile([128, 256], BF, tag="warm")
        nc.gpsimd.memset(warm[:], 0.0)
        wps = ppm.tile([128, H], F32, tag="x")
        for _ in range(10):
            nc.tensor.matmul(wps[:, 0:256], warm[:, 0:128], warm[:],
                             start=True, stop=True)

        src_t = encT.rearrange("p (t x) -> p t x", t=NT)

        def enc_tile(n):
            return encp.tile([128, n, HC * 128], ENC_DT, tag="enc",
                             name="enc_t")

        def enc_load(g):
            e_t = enc_tile(SLOTS[g])
            lo = S_OFF[g]
            nc.sync.dma_start(e_t[:], src_t[:, lo:lo + SLOTS[g], :])
            return e_t

        # slot 0's enc via Pool SWDGE
        e0 = enc_tile(SLOTS[0])
        nc.gpsimd.dma_start(e0[:], src_t[:, 0:SLOTS[0], :])
        PREFETCH = 5
        pre = {0: e0}
        for g in range(1, min(PREFETCH, NSLOT)):
            pre[g] = enc_load(g)

        vbc_sb = consts.tile([128, H], BF, tag="vbc")
        nc.gpsimd.dma_start(vbc_sb[:], vbc[:])

        att_t = consts.tile([128, NT], F32, tag="att")

        r1l4 = r1_sb[:, 0:NT * 2 * 128] \
            .rearrange("p (t x c) -> p t x c", t=NT, x=2)
        r1r4 = r1_sb[:, NT * 2 * 128:] \
            .rearrange("p (t x k) -> p t x k", t=NT, x=2)
        wb3a = wb_a[:].rearrange("p (c k) -> p c k", c=2)
        wb3b = wb_b[:].rearrange("p (c k) -> p c k", c=2)

        pend = None
        for g in range(NSLOT):
            NQ = SLOTS[g]
            T0 = S_OFF[g]
            enc_t = pre.pop(g)
            if g + PREFETCH < NSLOT:
                pre[g + PREFETCH] = enc_load(g + PREFETCH)

            enc4 = enc_t[:].rearrange("p q (c y) -> p q c y", c=HC)
            ps = ppm.tile([128, NQ * H], F32, tag="x")
            # rank-1s of all tiles first: they depend only on the small r1
            # blob, so at the head PE starts (and ramps) before enc lands
            for q in range(NQ):
                t = T0 + q
                nc.tensor.matmul(
                    ps[:, q * H:(q + 1) * H],
                    r1l4[:, t],
                    r1r4[:, t],
                    start=True, stop=False, perf_mode=DR,
                )
            for q in range(NQ):
                psl = ps[:, q * H:(q + 1) * H]
                for c, wb in ((0, wb3a), (2, wb3b)):
                    nc.tensor.matmul(
                        psl,
                        enc4[:, q, c:c + 2, :],
                        wb[:, 0:2, :],
                        start=False, stop=(c + 2 == HC),
                        perf_mode=DR,
                    )

            # ACT/DVE granularity: whole slot (one tanh / one mult instr
            # amortizes the access-latency init over the slot)
            if g == NSLOT - 1:
                pieces = [(0, NQ - 1), (NQ - 1, 1)] if NQ > 1 else [(0, 1)]
            else:
                pieces = [(0, NQ)]
            t_t = tpool.tile([128, NQ * H], BF, tag="t")
            scr = spool.tile([128, NQ * H], BF, tag="scr")
            for g0, glen in pieces:
                sl = slice(g0 * H, (g0 + glen) * H)
                nc.scalar.activation(t_t[:, sl], ps[:, sl], AF.Tanh,
                                     scale=1.0 / PSUM_SCALE)
                vb = vbc_sb[:].unsqueeze(1).broadcast_to([128, glen, H])
                nc.vector.tensor_tensor(
                    scr[:, sl].rearrange("p (q k) -> p q k", q=glen),
                    t_t[:, sl].rearrange("p (q k) -> p q k", q=glen),
                    vb, OP.mult)
                # defer the last reduce of a full slot to just after the NEXT
                # slot's tensor_tensor: DVE then has ready work queued while
                # the next tanh's semaphore resolves, instead of stalling
                for q in range(g0, g0 + glen):
                    if pend is not None:
                        ps_, qs_, te_ = pend
                        scr2 = s2pool.tile([128, H], BF, tag="scr2")
                        nc.vector.tensor_scalar(
                            scr2[:], ps_[:, qs_ * H:(qs_ + 1) * H], 1.0, None,
                            OP.mult, OP.add, accum_out=att_t[:, te_:te_ + 1],
                        )
                        pend = None
                    t = T0 + q
                    if glen == NQ == 4 and q == g0 + glen - 1 and \
                            g < NSLOT - 3:
                        pend = (scr, q, t)
                        continue
                    scr2 = s2pool.tile([128, H], BF, tag="scr2")
                    nc.vector.tensor_scalar(
                        scr2[:], scr[:, q * H:(q + 1) * H], 1.0, None,
                        OP.mult, OP.add, accum_out=att_t[:, t:t + 1],
                    )
            # ship logits in pieces so only the last columns' DMA trails the
            # final compute
            if g == NSLOT - 4:
                nc.sync.dma_start(att_out[:, 0:T0 + NQ], att_t[:, 0:T0 + NQ])
            elif g == NSLOT - 2:
                nc.sync.dma_start(att_out[:, S_OFF[NSLOT - 4] + SLOTS[NSLOT - 4]:T0 + NQ],
                                  att_t[:, S_OFF[NSLOT - 4] + SLOTS[NSLOT - 4]:T0 + NQ])
        lastT = S_OFF[NSLOT - 1]
        nc.sync.dma_start(att_out[:, lastT:], att_t[:, lastT:])

    nc.compile()
    return nc


def _get_nc(NT=None):
    if NT is None:
        NT = _CACHE.get("last_nt")
        assert NT is not None, "call kernel() first"
    if ("nc", NT) not in _CACHE:
        _CACHE[("nc", NT)] = _build_nc(NT)
    _CACHE["last_nt"] = NT
    return _CACHE[("nc", NT)]


def _prep(dec_input, enc_output, text_lengths, coverage_vector, W, b, v_w):
    enc = np.asarray(enc_output, dtype=np.float32)
    dec = np.asarray(dec_input, dtype=np.float32).reshape(B, E)
    cov = np.asarray(coverage_vector, dtype=np.float32)
    W = np.asarray(W, dtype=np.float32)
    b = np.asarray(b, dtype=np.float32)
    v_w = np.asarray(v_w, dtype=np.float32)
    lens = np.asarray(text_lengths).astype(np.int64)

    # ---- position-level tile packing ----
    # Full 128-position tiles per batch; the 32 partial tails are bin-packed
    # pairwise (<=2 batch segments per tile, handled by the K=4 rank-1).
    nfull = (lens // 128).astype(int)
    tail = (lens - nfull * 128).astype(int)

    tiles = []  # each: list of (b, s_start, n)
    for bi in range(B):
        for j in range(nfull[bi]):
            tiles.append([(bi, j * 128, 128)])
    tails = sorted([(int(tail[bi]), bi) for bi in range(B) if tail[bi] > 0],
                   reverse=True)
    open_tiles = []  # (free, idx)
    for n, bi in tails:
        placed = False
        for k, (free, idx) in enumerate(open_tiles):
            if free >= n and len(tiles[idx]) < 2:
                tiles[idx].append((bi, nfull[bi] * 128, n))
                open_tiles[k] = (free - n, idx)
                placed = True
                break
        if not placed:
            tiles.append([(bi, nfull[bi] * 128, n)])
            open_tiles.append((128 - n, len(tiles) - 1))

    T_real = len(tiles)
    NT = max(16, -(-T_real // NCORES))
    NT += NT % 2                          # slot pattern needs an even count
    T_pad = NT * NCORES
    tiles += [[(0, 0, 128)]] * (T_pad - T_real)

    # position-level index arrays
    bb = np.zeros((T_pad, 128), np.int64)     # batch of each position
    ss = np.zeros((T_pad, 128), np.int64)     # s of each position
    valid = np.zeros((T_pad, 128), bool)
    seg2 = np.zeros((T_pad, 128), bool)       # position belongs to segment 2
    b1 = np.zeros(T_pad, np.int64)
    b2 = np.zeros(T_pad, np.int64)
    for t, segs in enumerate(tiles):
        off = 0
        b1[t] = segs[0][0]
        b2[t] = segs[-1][0]
        for si, (bi, s0, n) in enumerate(segs):
            bb[t, off:off + n] = bi
            ss[t, off:off + n] = s0 + np.arange(n)
            valid[t, off:off + n] = True
            if si == 1:
                seg2[t, off:off + n] = True
            off += n

    # enc gather+transpose for selected positions only:
    # block[p, t, c*128+si] = enc[bb[t,si], ss[t,si], 128c+p] * ENC_SCALE
    blk = enc[bb, ss]                           # [T, si, h]
    blk = (blk * ENC_SCALE).astype(FP8).reshape(T_pad, 128, HC, 128)
    encT = np.ascontiguousarray(blk.transpose(3, 0, 2, 1)) \
        .reshape(128, T_pad * HC * 128)

    wenc = W[:H] * W_SCALE
    wblob = np.ascontiguousarray(
        wenc.reshape(HC, 128, H).transpose(1, 0, 2).reshape(128, HC * H)
    ).astype(FP8)

    dec_proj = dec @ W[H:H + E] + b                    # (B, H)
    wcovsum = W[H + E:].sum(axis=0, dtype=np.float32)  # (H,)

    # rank-1 blobs, fp8 DR: cov centered at 0.5 to halve fp8 quantization
    # error; the 0.5*w shift folds into u (kept exact in fp32 before cast)
    covp = cov[bb, ss] - 0.5
    m1 = (valid & ~seg2).astype(np.float32)
    m2 = (valid & seg2).astype(np.float32)
    r1l = np.empty((2, T_pad, 2, 128), np.float32)
    r1l[0, :, 0] = m1
    r1l[0, :, 1] = covp * m1
    r1l[1, :, 0] = m2
    r1l[1, :, 1] = covp * m2
    uu = (dec_proj + 0.5 * wcovsum) * PSUM_SCALE       # (B, H)
    r1r = np.empty((2, T_pad, 2, H), np.float32)
    r1r[0, :, 0] = uu[b1]
    r1r[0, :, 1] = wcovsum * PSUM_SCALE
    r1r[1, :, 0] = uu[b2]
    r1r[1, :, 1] = wcovsum * PSUM_SCALE
    r1l = r1l.astype(FP8)
    r1r = r1r.astype(FP8)

    vbc = np.ascontiguousarray(np.broadcast_to(v_w.astype(BF16), (128, H)))

    in_maps = []
    for core in range(NCORES):
        sl = slice(core * NT, (core + 1) * NT)
        in_maps.append({
            "encT": np.ascontiguousarray(
                encT.reshape(128, T_pad, HC * 128)[:, sl]
                .reshape(128, NT * HC * 128)),
            "wblob": wblob,
            "r1f": np.ascontiguousarray(np.concatenate(
                [r1l[:, sl].reshape(2, NT * 2 * 128),
                 r1r[:, sl].reshape(2, NT * 2 * H)], axis=1)),
            "vbc": vbc,
        })
    return in_maps, bb, ss, valid, NT


def kernel(dec_input, enc_output, text_lengths, coverage_vector, W, b, v_w, v_b):
    from concourse.bass_utils import run_bass_kernel_spmd

    in_maps, bb, ss, valid, NT = _prep(
        dec_input, enc_output, text_lengths, coverage_vector, W, b, v_w)
    nc = _get_nc(NT)
    res = run_bass_kernel_spmd(nc, in_maps, core_ids=list(range(NCORES)))

    # scatter raw logits back to (b, s); untouched positions stay -inf
    logits = np.full((B, S), -np.inf, np.float32)
    cols = np.concatenate([res.results[c]["att_out"].T for c in range(NCORES)],
                          axis=0)                     # [T_pad, 128]
    logits[bb[valid], ss[valid]] = cols[valid]
    # masked softmax epilogue (full fp32, max-subtracted)
    lens = np.asarray(text_lengths).reshape(B, 1)
    masked = np.where(np.arange(S)[None, :] < lens, logits, -np.inf)
    masked -= masked.max(axis=1, keepdims=True)
    att = np.exp(masked)
    att /= att.sum(axis=1, keepdims=True, dtype=np.float32)
    ncov = np.asarray(coverage_vector, dtype=np.float32) + att
    return att, ncov


# revision 28
# speedup vs baseline: 1.0596x; 1.0028x over previous
"""Fused sparse-attention kernel for Trainium2 (8 NeuronCores).

Computation (per batch element b):
    X[s,k]  = enc[b] @ W_enc + dec_proj[b,k] + cov[b,s]*Wcovsum[k] + bias[k]
    T       = tanh(X)
    att[s]  = T @ v_w                      (+ v_b, which cancels in softmax)
    w       = softmax(att masked to s < len[b])
    new_cov = cov + w

Key insight vs the batch-parallel baseline: positions s >= text_lengths[b]
are masked to -inf, so their softmax weight is exactly 0 and new_cov equals
cov there.  Only ceil(len_b/128) of the 16 s-tiles per batch need computing
(~55% on average for uniform lengths).  The device work unit is therefore a
flat list of (b, j) 128-position s-tiles packed by the host; tiles are dealt
round-robin to the 8 cores (a batch may straddle cores -- the softmax is a
host epilogue, so tiles are fully independent).  Every engine's load scales
with the masked tile count.

Per-tile pipeline (same numerics as the proven baseline):
  PE:  X psum[128s, 512k] = fp8 DoubleRow GEMM (enc fp8 *0.25, W_enc fp8 *16,
       net *4 undone by tanh's scale) + bf16 K=2 rank-1 (ones,cov) x (u_b, w)
  ACT: tanh over a whole 4-tile psum slot -> bf16 (one instr amortizes the
       ~185ns PSUM/SBUF access init)
  DVE: slot-wide tensor_tensor T*v (2x bf16), then per-tile tensor_scalar
       with accum_out for the free-dim reduce (4x mode)
PSUM rotates 2 slots x 4 banks.  Raw logits ship to the host, which does the
masked softmax (fp32, max-subtracted) + cov add on 65K values.

The Bass program depends only on NT (padded tiles per core), compiled on
first call per NT and cached; all (b,j) specifics live in host-packed input
blobs, so any text_lengths works.
"""

import numpy as np
import ml_dtypes

B, S, H, E = 32, 2048, 512, 512
NCORES = 8
HC = H // 128               # h chunks
BF16 = ml_dtypes.bfloat16
FP8 = ml_dtypes.float8_e4m3fn
ENC_SCALE = 0.25            # enc pre-scale (host)
W_SCALE = 16.0              # W_enc pre-scale (host)
PSUM_SCALE = ENC_SCALE * W_SCALE  # net scale on psum; undone in tanh

_CACHE = {}


def _build_nc(NT):
    """NT = padded tile count per core (multiple of 4)."""
    import concourse.mybir as mybir
    import concourse.tile as tile
    from concourse import bacc
    from contextlib import ExitStack

    dt = mybir.dt
    F32, BF = dt.float32, dt.bfloat16
    ENC_DT = dt.float8e4

    nc = bacc.Bacc("TRN2", target_bir_lowering=False, debug=False,
                   enable_asserts=False, num_devices=NCORES)

    # ---- DRAM I/O (per-core shapes) ----
    # encT[p, t*512 + c*128 + si] = enc[b_t, 128*j_t + si, 128*c + p] * 0.25
    # (fp8): per-partition 512B-contiguous runs per tile => full DMA rate
    encT = nc.dram_tensor("encT", [128, NT * HC * 128], ENC_DT,
                          kind="ExternalInput").ap()
    # wblob: wenc chunk c at cols [c*H,(c+1)*H): wenc[c][p,k] = W[128c+p, k]
    wblob = nc.dram_tensor("wblob", [128, HC * H], ENC_DT,
                           kind="ExternalInput").ap()
    # rank-1 blobs, fp8 DoubleRow layout (K=4 = 2 partitions x 2 DR rows) so a
    # tile can mix positions from two batches (the 32 partial batch tails are
    # bin-packed pairwise into shared tiles).  Per tile:
    #   lhs [2p, 2x, 128s] = ((m1, cov'*m1), (m2, cov'*m2)),  cov' = cov-0.5
    #   rhs [2p, 2x, 512k] = ((u_b1', w), (u_b2', w)) * PSUM_SCALE,
    #   u' = dec_proj + bias + 0.5*w  (the 0.5 undoes the cov centering)
    r1f = nc.dram_tensor("r1f", [2, NT * 2 * (128 + H)], ENC_DT,
                         kind="ExternalInput").ap()
    vbc = nc.dram_tensor("vbc", [128, H], BF, kind="ExternalInput").ap()
    # raw attention logits, column t = tile t's 128 s-positions
    att_out = nc.dram_tensor("att_out", [128, NT], F32,
                             kind="ExternalOutput").ap()

    AF = mybir.ActivationFunctionType
    OP = mybir.AluOpType
    DR = mybir.MatmulPerfMode.DoubleRow

    # slot sizes: small slots at both ends shorten pipeline fill (first tanh
    # waits on a whole psum-tile slot: sync is tile-granular) and drain
    if NT >= 16:
        mid = NT - 8
        SLOTS = [1, 1, 2] + [4] * (mid // 4) + ([2] if mid % 4 else []) \
            + [2, 1, 1]
    else:
        SLOTS = [2] * (NT // 2)
    assert sum(SLOTS) == NT
    NSLOT = len(SLOTS)
    S_OFF = [sum(SLOTS[:i]) for i in range(NSLOT)]

    with tile.TileContext(nc) as tc, ExitStack() as ctx:
        consts = ctx.enter_context(tc.tile_pool(name="consts", bufs=1))
        encp = ctx.enter_context(tc.tile_pool(name="encp", bufs=6))
        tpool = ctx.enter_context(tc.tile_pool(name="tpool", bufs=4))
        spool = ctx.enter_context(tc.tile_pool(name="spool", bufs=4))
        s2pool = ctx.enter_context(tc.tile_pool(name="s2pool", bufs=5))
        ppm = ctx.enter_context(tc.tile_pool(name="ppm", bufs=2, space="PSUM"))

        # first-needed consts ride the SP HWDGE queue, smallest first; slot 0's
        # enc takes the Pool SWDGE path whose desc-gen overlaps the serialized
        # HWDGE issues.
        r1_sb = consts.tile([2, NT * 2 * (128 + H)], ENC_DT, tag="r1f")
        nc.sync.dma_start(r1_sb[:], r1f[:])
        wb_a = consts.tile([128, 2 * H], ENC_DT, tag="wb_a")
        nc.sync.dma_start(wb_a[:], wblob[:, 0:2 * H])
        wb_b = consts.tile([128, 2 * H], ENC_DT, tag="wb_b")
        nc.sync.dma_start(wb_b[:], wblob[:, 2 * H:])

        # PE p-state warm-up: junk matmuls on memset scratch keep the tensor
        # engine continuously busy from ~1us so the 3us ramp to full clock is
        # mostly done before the first real matmul (otherwise the whole
        # pipeline fill runs at the 2-4x slower mid/low p-state)
        warm = consts.tile([128, 256], BF, tag="warm")
        nc.gpsimd.memset(warm[:], 0.0)
        wps = ppm.tile([128, H], F32, tag="x")
        for _ in range(10):
            nc.tensor.matmul(wps[:, 0:256], warm[:, 0:128], warm[:],
                             start=True, stop=True)

        src_t = encT.rearrange("p (t x) -> p t x", t=NT)

        def enc_tile(n):
            return encp.tile([128, n, HC * 128], ENC_DT, tag="enc",
                             name="enc_t")

        def enc_load(g):
            e_t = enc_tile(SLOTS[g])
            lo = S_OFF[g]
            nc.sync.dma_start(e_t[:], src_t[:, lo:lo + SLOTS[g], :])
            return e_t

        # slot 0's enc via Pool SWDGE
        e0 = enc_tile(SLOTS[0])
        nc.gpsimd.dma_start(e0[:], src_t[:, 0:SLOTS[0], :])
        PREFETCH = 5
        pre = {0: e0}
        for g in range(1, min(PREFETCH, NSLOT)):
            pre[g] = enc_load(g)

        vbc_sb = consts.tile([128, H], BF, tag="vbc")
        nc.gpsimd.dma_start(vbc_sb[:], vbc[:])

        att_t = consts.tile([128, NT], F32, tag="att")

        r1l4 = r1_sb[:, 0:NT * 2 * 128] \
            .rearrange("p (t x c) -> p t x c", t=NT, x=2)
        r1r4 = r1_sb[:, NT * 2 * 128:] \
            .rearrange("p (t x k) -> p t x k", t=NT, x=2)
        wb3a = wb_a[:].rearrange("p (c k) -> p c k", c=2)
        wb3b = wb_b[:].rearrange("p (c k) -> p c k", c=2)

        pend = None
        for g in range(NSLOT):
            NQ = SLOTS[g]
            T0 = S_OFF[g]
            enc_t = pre.pop(g)
            if g + PREFETCH < NSLOT:
                pre[g + PREFETCH] = enc_load(g + PREFETCH)

            enc4 = enc_t[:].rearrange("p q (c y) -> p q c y", c=HC)
            ps = ppm.tile([128, NQ * H], F32, tag="x")
            # rank-1s of all tiles first: they depend only on the small r1
            # blob, so at the head PE starts (and ramps) before enc lands
            for q in range(NQ):
                t = T0 + q
                nc.tensor.matmul(
                    ps[:, q * H:(q + 1) * H],
                    r1l4[:, t],
                    r1r4[:, t],
                    start=True, stop=False, perf_mode=DR,
                )
            for q in range(NQ):
                psl = ps[:, q * H:(q + 1) * H]
                for c, wb in ((0, wb3a), (2, wb3b)):
                    nc.tensor.matmul(
                        psl,
                        enc4[:, q, c:c + 2, :],
                        wb[:, 0:2, :],
                        start=False, stop=(c + 2 == HC),
                        perf_mode=DR,
                    )

            # ACT/DVE granularity: whole slot (one tanh / one mult instr
            # amortizes the access-latency init over the slot)
            if g == NSLOT - 1:
                pieces = [(0, NQ - 1), (NQ - 1, 1)] if NQ > 1 else [(0, 1)]
            else:
                pieces = [(0, NQ)]
            t_t = tpool.tile([128, NQ * H], BF, tag="t")
            scr = spool.tile([128, NQ * H], BF, tag="scr")
            for g0, glen in pieces:
                sl = slice(g0 * H, (g0 + glen) * H)
                nc.scalar.activation(t_t[:, sl], ps[:, sl], AF.Tanh,
                                     scale=1.0 / PSUM_SCALE)
                vb = vbc_sb[:].unsqueeze(1).broadcast_to([128, glen, H])
                nc.vector.tensor_tensor(
                    scr[:, sl].rearrange("p (q k) -> p q k", q=glen),
                    t_t[:, sl].rearrange("p (q k) -> p q k", q=glen),
                    vb, OP.mult)
                # defer the last reduce of a full slot to just after the NEXT
                # slot's tensor_tensor: DVE then has ready work queued while
                # the next tanh's semaphore resolves, instead of stalling
                for q in range(g0, g0 + glen):
                    if pend is not None:
                        ps_, qs_, te_ = pend
                        scr2 = s2pool.tile([128, H], BF, tag="scr2")
                        nc.vector.tensor_scalar(
                            scr2[:], ps_[:, qs_ * H:(qs_ + 1) * H], 1.0, None,
                            OP.mult, OP.add, accum_out=att_t[:, te_:te_ + 1],
                        )
                        pend = None
                    t = T0 + q
                    if glen == NQ == 4 and q == g0 + glen - 1 and \
                            g < NSLOT - 3:
                        pend = (scr, q, t)
                        continue
                    scr2 = s2pool.tile([128, H], BF, tag="scr2")
                    nc.vector.tensor_scalar(
                        scr2[:], scr[:, q * H:(q + 1) * H], 1.0, None,
                        OP.mult, OP.add, accum_out=att_t[:, t:t + 1],
                    )
            # ship logits in pieces so only the last columns' DMA trails the
            # final compute
            if g == NSLOT - 4:
                nc.sync.dma_start(att_out[:, 0:T0 + NQ], att_t[:, 0:T0 + NQ])
            elif g == NSLOT - 2:
                nc.sync.dma_start(att_out[:, S_OFF[NSLOT - 4] + SLOTS[NSLOT - 4]:T0 + NQ],
                                  att_t[:, S_OFF[NSLOT - 4] + SLOTS[NSLOT - 4]:T0 + NQ])
        lastT = S_OFF[NSLOT - 1]
        nc.sync.dma_start(att_out[:, lastT:], att_t[:, lastT:])

    nc.compile()
    return nc


def _get_nc(NT=None):
    if NT is None:
        NT = _CACHE.get("last_nt")
        assert NT is not None, "call kernel() first"
    if ("nc", NT) not in _CACHE:
        _CACHE[("nc", NT)] = _build_nc(NT)
    _CACHE["last_nt"] = NT
    return _CACHE[("nc", NT)]


def _prep(dec_input, enc_output, text_lengths, coverage_vector, W, b, v_w):
    enc = np.asarray(enc_output, dtype=np.float32)
    dec = np.asarray(dec_input, dtype=np.float32).reshape(B, E)
    cov = np.asarray(coverage_vector, dtype=np.float32)
    W = np.asarray(W, dtype=np.float32)
    b = np.asarray(b, dtype=np.float32)
    v_w = np.asarray(v_w, dtype=np.float32)
    lens = np.asarray(text_lengths).astype(np.int64)

    # ---- position-level tile packing ----
    # Full 128-position tiles per batch; the 32 partial tails are bin-packed
    # pairwise (<=2 batch segments per tile, handled by the K=4 rank-1).
    nfull = (lens // 128).astype(int)
    tail = (lens - nfull * 128).astype(int)

    tiles = []  # each: list of (b, s_start, n)
    for bi in range(B):
        for j in range(nfull[bi]):
            tiles.append([(bi, j * 128, 128)])
    tails = sorted([(int(tail[bi]), bi) for bi in range(B) if tail[bi] > 0],
                   reverse=True)
    open_tiles = []  # (free, idx)
    for n, bi in tails:
        placed = False
        for k, (free, idx) in enumerate(open_tiles):
            if free >= n and len(tiles[idx]) < 2:
                tiles[idx].append((bi, nfull[bi] * 128, n))
                open_tiles[k] = (free - n, idx)
                placed = True
                break
        if not placed:
            tiles.append([(bi, nfull[bi] * 128, n)])
            open_tiles.append((128 - n, len(tiles) - 1))

    T_real = len(tiles)
    NT = max(16, -(-T_real // NCORES))
    NT += NT % 2                          # slot pattern needs an even count
    T_pad = NT * NCORES
    tiles += [[(0, 0, 128)]] * (T_pad - T_real)

    # position-level index arrays
    bb = np.zeros((T_pad, 128), np.int64)     # batch of each position
    ss = np.zeros((T_pad, 128), np.int64)     # s of each position
    valid = np.zeros((T_pad, 128), bool)
    seg2 = np.zeros((T_pad, 128), bool)       # position belongs to segment 2
    b1 = np.zeros(T_pad, np.int64)
    b2 = np.zeros(T_pad, np.int64)
    for t, segs in enumerate(tiles):
        off = 0
        b1[t] = segs[0][0]
        b2[t] = segs[-1][0]
        for si, (bi, s0, n) in enumerate(segs):
            bb[t, off:off + n] = bi
            ss[t, off:off + n] = s0 + np.arange(n)
            valid[t, off:off + n] = True
            if si == 1:
                seg2[t, off:off + n] = True
            off += n

    # enc gather+transpose for selected positions only:
    # block[p, t, c*128+si] = enc[bb[t,si], ss[t,si], 128c+p] * ENC_SCALE
    blk = enc[bb, ss]                           # [T, si, h]
    blk = (blk * ENC_SCALE).astype(FP8).reshape(T_pad, 128, HC, 128)
    encT = np.ascontiguousarray(blk.transpose(3, 0, 2, 1)) \
        .reshape(128, T_pad * HC * 128)

    wenc = W[:H] * W_SCALE
    wblob = np.ascontiguousarray(
        wenc.reshape(HC, 128, H).transpose(1, 0, 2).reshape(128, HC * H)
    ).astype(FP8)

    dec_proj = dec @ W[H:H + E] + b                    # (B, H)
    wcovsum = W[H + E:].sum(axis=0, dtype=np.float32)  # (H,)

    # rank-1 blobs, fp8 DR: cov centered at 0.5 to halve fp8 quantization
    # error; the 0.5*w shift folds into u (kept exact in fp32 before cast)
    covp = cov[bb, ss] - 0.5
    m1 = (valid & ~seg2).astype(np.float32)
    m2 = (valid & seg2).astype(np.float32)
    r1l = np.empty((2, T_pad, 2, 128), np.float32)
    r1l[0, :, 0] = m1
    r1l[0, :, 1] = covp * m1
    r1l[1, :, 0] = m2
    r1l[1, :, 1] = covp * m2
    uu = (dec_proj + 0.5 * wcovsum) * PSUM_SCALE       # (B, H)
    r1r = np.empty((2, T_pad, 2, H), np.float32)
    r1r[0, :, 0] = uu[b1]
    r1r[0, :, 1] = wcovsum * PSUM_SCALE
    r1r[1, :, 0] = uu[b2]
    r1r[1, :, 1] = wcovsum * PSUM_SCALE
    r1l = r1l.astype(FP8)
    r1r = r1r.astype(FP8)

    vbc = np.ascontiguousarray(np.broadcast_to(v_w.astype(BF16), (128, H)))

    in_maps = []
    for core in range(NCORES):
        sl = slice(core * NT, (core + 1) * NT)
        in_maps.append({
            "encT": np.ascontiguousarray(
                encT.reshape(128, T_pad, HC * 128)[:, sl]
                .reshape(128, NT * HC * 128)),
            "wblob": wblob,
            "r1f": np.ascontiguousarray(np.concatenate(
                [r1l[:, sl].reshape(2, NT * 2 * 128),
                 r1r[:, sl].reshape(2, NT * 2 * H)], axis=1)),
            "vbc": vbc,
        })
    return in_maps, bb, ss, valid, NT


def kernel(dec_input, enc_output, text_lengths, coverage_vector, W, b, v_w, v_b):
    from concourse.bass_utils import run_bass_kernel_spmd

    in_maps, bb, ss, valid, NT = _prep(
        dec_input, enc_output, text_lengths, coverage_vector, W, b, v_w)
    nc = _get_nc(NT)
    res = run_bass_kernel_spmd(nc, in_maps, core_ids=list(range(NCORES)))

    # scatter raw logits back to (b, s); untouched positions stay -inf
    logits = np.full((B, S), -np.inf, np.float32)
    cols = np.concatenate([res.results[c]["att_out"].T for c in range(NCORES)],
                          axis=0)                     # [T_pad, 128]
    logits[bb[valid], ss[valid]] = cols[valid]
    # masked softmax epilogue (full fp32, max-subtracted)
    lens = np.asarray(text_lengths).reshape(B, 1)
    masked = np.where(np.arange(S)[None, :] < lens, logits, -np.inf)
    masked -= masked.max(axis=1, keepdims=True)
    att = np.exp(masked)
    att /= att.sum(axis=1, keepdims=True, dtype=np.float32)
    ncov = np.asarray(coverage_vector, dtype=np.float32) + att
    return att, ncov
